# revision 63
# baseline (speedup 1.0000x reference)
"""Multi-head attention (B=2, S=2048, D=1024, H=16, DK=64) on 8 Trainium2 cores.

Sharding: 8 cores x (1 batch, 4 heads) each.  Core c handles batch c//4,
heads [4*(c%4) : 4*(c%4)+4].  Each core computes its heads' slice of the
output projection (rows of Wo for its heads); the host sums the 4 partial
outputs per batch and adds the bias.

Per-core dataflow (all matmul inputs bf16, PSUM accumulation fp32):
  - host supplies q/k/v pre-transposed per batch (qT/kT/vT [D, S]) and
    Wq pre-scaled by 1/sqrt(dk); tensors stream in 512-column chunks
    (wk/kT0 kc-granular) so projections start while DMA is in flight
  - qhT/khT [dk2=128, S] per head-pair via PE (weights stationary); the
    pair's two heads live in partition halves 0:64 / 64:128
  - vh natural [S, dk4] via PE (one N=256 group covers all 4 heads),
    with a ones column appended per head for softmax row sums
  - attention runs per (pair, 512-wide q chunk): for each key chunk mc,
    the two heads' scoresT[m, q] matmuls (K=64) are issued back-to-back
    into PE row-groups (0,0) and (64,0) -- they execute CONCURRENTLY on
    disjoint 64-row halves of the PE array, doubling scores throughput
  - one ACT exp instruction per mc covers both heads' [128, 1024] scores
    (no max-subtract: |scores/8| < ~6), writing bf16 attnT
  - outT(+sums) = vh_aug.T-chunk @ attnT accumulated over mc into a
    [65, 512] PSUM bank per head, lagging the exp stream by 3 so the PE
    stays dense across unit boundaries while ACT catches up
  - normalize: pout is copied to SBUF first (freeing the PSUM slot for
    the next unit's attn@v), then reciprocal + broadcast multiply on
    DVE/GpSimd off the critical path
  - partial output (bf16) = outT2.T-chunk @ Wo-rows accumulated over
    head pairs; both 512-wide halves share one staging tile + DMA

The PE is the critical engine (~176us busy vs ~137us of ACT exp), so
all projection and output-projection work drains as fill items inside
the attention iterations, scheduled so no unit starves: unit 0 absorbs
vh + the rest of pair-0's projections at 2 fills/iter, units 4-6 get
pair-1's khT/qhT (emitted just ahead of their consumers), and outproj
items join at iters 2/12 of the unit after their q-range's normalize.
The last unit's normalize+outproj interleave column-chunked so the PE
never idles long enough for HAM to re-throttle during the drain.
"""

import numpy as np
import ml_dtypes
from contextlib import ExitStack

import concourse.bass as bass
import concourse.tile as tile
from concourse import bacc, mybir
from concourse import bass_utils

B, S, D, H, DK = 2, 2048, 1024, 16, 64
N_CORES = 8
HPC = 4            # heads per core
PAIRS = HPC // 2   # head pairs per core
KC = D // 128      # contraction chunks over D
MC = S // 128      # m (key) chunks
SC = S // 512      # 512-wide column chunks of S
F32 = mybir.dt.float32
BF16 = mybir.dt.bfloat16
BF16_NP = ml_dtypes.bfloat16

_COMPILED = {}


def _emit(tc, qT, kT, vT, wq, wk, wv, wo, out_dram):
    nc = tc.nc
    AFT = mybir.ActivationFunctionType
    qTa, kTa, vTa = qT.ap(), kT.ap(), vT.ap()
    wqa, wka, wva, woa = wq.ap(), wk.ap(), wv.ap(), wo.ap()
    outa = out_dram.ap()

    with ExitStack() as ctx:
        big = ctx.enter_context(tc.tile_pool(name="big", bufs=1))
        att = ctx.enter_context(tc.tile_pool(name="att", bufs=10))
        dance = ctx.enter_context(tc.tile_pool(name="dance", bufs=2))
        ostage = ctx.enter_context(tc.tile_pool(name="ostage", bufs=4))
        # PSUM budget (8 banks): scores ping-pong 2x[128,1024] (4 banks),
        # attn@v accumulators 2x[65,512] (2 banks), proj/outproj fill
        # accumulators 2x[128,512] (2 banks)
        ppool = ctx.enter_context(tc.tile_pool(name="psum_s", bufs=2, space="PSUM"))
        vpool = ctx.enter_context(tc.tile_pool(name="psum_v", bufs=2, space="PSUM"))
        fpool = ctx.enter_context(tc.tile_pool(name="psum_f", bufs=2, space="PSUM"))

        # ---- weights first (small), then 512-col chunks of kT/vT/qT ------
        wq_sb = big.tile([128, KC, HPC * DK], BF16, tag="wq")
        wk_sb = big.tile([128, KC, HPC * DK], BF16, tag="wk")
        wv_sb = big.tile([128, KC, HPC * DK], BF16, tag="wv")
        wo_sb = big.tile([128, PAIRS, D], BF16, tag="wo")


        # warm the ACT exp table during the DMA phase
        warm_sb = big.tile([1, 64], BF16, tag="warm")
        nc.vector.memset(warm_sb[:], 1.0)
        nc.scalar.activation(warm_sb[:], warm_sb[:], AFT.Exp)

        # chunked tensor tiles: kT/qT/vT in SC chunks of [128, KC, 512] so
        # each projection group only waits for the chunk it reads
        def chunk_tiles(name):
            return [
                big.tile([128, KC, 512], BF16, tag=f"{name}{s}", name=f"{name}{s}")
                for s in range(SC)
            ]

        kT_sb, qT_sb, vT_sb = chunk_tiles("kT"), chunk_tiles("qT"), chunk_tiles("vT")

        def dma_chunk(dst, src, s):
            nc.sync.dma_start(
                dst[s][:],
                src[:, s * 512:(s + 1) * 512].rearrange("(c p) s -> p c s", p=128),
            )

        # DMA order: feed the minimal upfront (khT sc0, qhT sc0) first so
        # the PE starts as early as possible; everything else streams in as
        # the fills consume it.  wk+kT0 interleave kc-granular so the very
        # first khT matmul needs only ~190KB of DMA, not 1.5MB.
        wka_r = wka.rearrange("(c p) n -> p c n", p=128)
        kTa_r0 = kTa[:, 0:512].rearrange("(c p) s -> p c s", p=128)
        for kc in range(KC):
            nc.sync.dma_start(wk_sb[:, kc:kc + 1, :], wka_r[:, kc:kc + 1, :])
            nc.sync.dma_start(kT_sb[0][:, kc:kc + 1, :], kTa_r0[:, kc:kc + 1, :])
        nc.sync.dma_start(wq_sb[:], wqa.rearrange("(c p) n -> p c n", p=128))
        dma_chunk(qT_sb, qTa, 0)
        nc.sync.dma_start(wv_sb[:], wva.rearrange("(c p) n -> p c n", p=128))
        dma_chunk(vT_sb, vTa, 0)
        dma_chunk(kT_sb, kTa, 1)
        dma_chunk(vT_sb, vTa, 1)
        dma_chunk(kT_sb, kTa, 2)
        dma_chunk(vT_sb, vTa, 2)
        dma_chunk(kT_sb, kTa, 3)
        dma_chunk(qT_sb, qTa, 1)
        dma_chunk(vT_sb, vTa, 3)
        dma_chunk(qT_sb, qTa, 2)
        dma_chunk(qT_sb, qTa, 3)
        nc.sync.dma_start(wo_sb[:], woa.rearrange("(c p) d -> p c d", p=128))

        # vh with a ones column per (m-chunk, head): [128, MC, HPC, 65]
        vh_sb = big.tile([128, MC, HPC, DK + 1], BF16, tag="vh")
        nc.vector.memset(vh_sb[:], 1.0)

        qhT_sb = [
            big.tile([128, S], BF16, tag=f"qhT{p}", name=f"qhT{p}")
            for p in range(PAIRS)
        ]
        khT_sb = [
            big.tile([128, S], BF16, tag=f"khT{p}", name=f"khT{p}")
            for p in range(PAIRS)
        ]
        outT2_sb = [
            big.tile([128, S], BF16, tag=f"o2{p}", name=f"o2{p}")
            for p in range(PAIRS)
        ]

        # ---- projection emitters ----------------------------------------
        def emit_proj_qk(p, w_sb, src, dst, s):
            """dst[:, s*512:+512] for pair p: 8 kc matmuls + copy."""
            ps = fpool.tile([128, 512], F32, tag="fp", name="ps_proj")
            for kc in range(KC):
                nc.tensor.matmul(
                    ps[:],
                    w_sb[:, kc, p * 128:(p + 1) * 128],
                    src[s][:, kc, :],
                    start=(kc == 0),
                    stop=(kc == KC - 1),
                )
            nc.vector.tensor_copy(dst[:, s * 512:(s + 1) * 512], ps[:])

        def proj_quarters(p, w_sb, src, dst, s):
            """emit_proj_qk split into 4 fill items of 2 matmuls each."""
            state = {}
            def quarter(i, state=state, p=p, w_sb=w_sb, src=src, dst=dst, s=s):
                if i == 0:
                    state["ps"] = fpool.tile([128, 512], F32, tag="fp", name="ps_fq")
                ps = state["ps"]
                for kc in range(2 * i, 2 * i + 2):
                    nc.tensor.matmul(
                        ps[:],
                        w_sb[:, kc, p * 128:(p + 1) * 128],
                        src[s][:, kc, :],
                        start=(kc == 0),
                        stop=(kc == KC - 1),
                    )
                if i == 3:
                    nc.vector.tensor_copy(dst[:, s * 512:(s + 1) * 512], ps[:])
            return [lambda i=i: quarter(i) for i in range(4)]

        def emit_proj_v(mc):
            """vh[:, mc, :, 0:DK] for all 4 heads: one fill item (N=256)."""
            s, col = divmod(mc, 4)
            ps = fpool.tile([128, 512], F32, tag="fp", name="ps_v")
            for kc in range(KC):
                nc.tensor.matmul(
                    ps[:, 0:256],
                    vT_sb[s][:, kc, col * 128:(col + 1) * 128],
                    wv_sb[:, kc, :],
                    start=(kc == 0),
                    stop=(kc == KC - 1),
                )
            nc.vector.tensor_copy(
                vh_sb[:, mc, :, 0:DK],
                ps[:, 0:256].rearrange("p (h k) -> p h k", k=DK),
            )

        def emit_outproj_half(qi, j, state={}):
            """Half j of output rows qi*128:+128; both halves share one
            staging tile and one DMA (fewer sync-engine triggers)."""
            ps = fpool.tile([128, 512], F32, tag="fp", name="ps_o")
            for p in range(PAIRS):
                nc.tensor.matmul(
                    ps[:],
                    outT2_sb[p][:, qi * 128:(qi + 1) * 128],
                    wo_sb[:, p, j * 512:(j + 1) * 512],
                    start=(p == 0),
                    stop=(p == PAIRS - 1),
                )
            if j == 0:
                state[qi] = ostage.tile([128, 1024], BF16, tag="so", name="so")
            so = state[qi]
            nc.vector.tensor_copy(so[:, j * 512:(j + 1) * 512], ps[:])
            if j == 1:
                nc.sync.dma_start(outa[qi * 128:(qi + 1) * 128, :], so[:])
                del state[qi]

        # ---- upfront PE work: only what unit 0's first iterations need ---
        emit_proj_qk(0, wk_sb, kT_sb, khT_sb[0], 0)
        emit_proj_qk(0, wq_sb, qT_sb, qhT_sb[0], 0)

        # ---- attention: flat software-pipelined stream -------------------
        def emit_av(st, mc):
            p = st["p"]
            at = st["at"][mc]
            for hh, pout in ((0, st["poutA"]), (1, st["poutB"])):
                nc.tensor.matmul(
                    pout[:],
                    vh_sb[:, mc, 2 * p + hh, :],
                    at[:, hh * 512:(hh + 1) * 512],
                    start=(mc == 0),
                    stop=(mc == MC - 1),
                )
            del st["at"][mc]

        def emit_dance(st):
            # copy each pout to SBUF FIRST: that read is all that gates the
            # next unit's attn@v reusing the pout PSUM slot; the normalize
            # chain below then runs off the critical path
            p, qc = st["p"], st["qc"]
            pcs = []
            for pout in (st["poutA"], st["poutB"]):
                pc = dance.tile([65, 512], F32, tag="pc", name="pc")
                nc.vector.tensor_copy(pc[:], pout[:])
                pcs.append(pc)
            for hh, pc in enumerate(pcs):
                hlo, hhi = hh * 64, hh * 64 + 64
                sums = dance.tile([1, 512], F32, tag="sums", name="sums")
                nc.vector.tensor_copy(sums[:], pc[64:65, :])
                rcp = dance.tile([1, 512], F32, tag="rcp", name="rcp")
                nc.vector.reciprocal_approx_fast(rcp[:], sums[:])
                rcpb = dance.tile([64, 512], F32, tag="rcpb", name="rcpb")
                nc.gpsimd.partition_broadcast(rcpb[:], rcp[:])
                nc.vector.tensor_tensor(
                    outT2_sb[p][hlo:hhi, qc * 512:(qc + 1) * 512],
                    pc[0:64, :],
                    rcpb[:],
                    mybir.AluOpType.mult,
                )

        def attention_unit(p, qc, fills, carry, pending=(), pending2=(),
                           unit0=False, lag=3):
            st = {
                "p": p, "qc": qc, "at": {},
                "poutA": vpool.tile([65, 512], F32, tag="po", name="poutA"),
                "poutB": vpool.tile([65, 512], F32, tag="po", name="poutB"),
            }
            qsl = slice(qc * 512, (qc + 1) * 512)

            def emit_scores(mc):
                # two heads' scoresT issued back-to-back into PE row-groups
                # (0,0) / (64,0): they run concurrently on the array halves
                ps = ppool.tile([128, 1024], F32, tag="pp", name="ps_sc")
                for hh in range(2):
                    nc.tensor.matmul(
                        ps[:, hh * 512:(hh + 1) * 512],
                        khT_sb[p][hh * 64:hh * 64 + 64,
                                  mc * 128:(mc + 1) * 128],
                        qhT_sb[p][hh * 64:hh * 64 + 64, qsl],
                        start=True,
                        stop=True,
                    )
                return ps

            # iterations grouped [0],[1],[2,3],...,[14,15]: mid-unit pairs
            # issue both iterations' scores back-to-back so pair mc+1's
            # array fill overlaps pair mc's drain (different row-groups);
            # the first two stay single so unit-boundary behavior (scores
            # gated on the previous unit's exp slot) is unchanged
            groups = [[0], [1]] + [[m, m + 1] for m in range(2, MC, 2)]
            for g in groups:
                pss = [emit_scores(m) for m in g]
                for m, ps in zip(g, pss):
                    at = att.tile([128, 1024], BF16, tag="attnT", name="at")
                    nc.scalar.activation(at[:], ps[:], AFT.Exp)
                    st["at"][m] = at
                for mc in g:
                    if mc == 2:
                        # items that depend on the carried dance having
                        # been emitted (outproj reading outT2) join only
                        # after the carry fully drained (iters 0-1)
                        fills.extend(pending)
                    if mc == 12:
                        fills.extend(pending2)
                    for _ in range(2):
                        if carry:
                            carry.pop(0)()
                    n_fill = 2 if (unit0 or mc < 2) else 1
                    for _ in range(n_fill):
                        if fills:
                            fills.pop(0)()
                    # attn@v last (lag 3): its vh LDWEIGHTS hides under
                    # the fill matmuls, and the deeper lag keeps the PE
                    # dense at unit boundaries while ACT catches up
                    if mc >= lag:
                        emit_av(st, mc - lag)
            return [
                (lambda mc=mc: emit_av(st, mc)) for mc in range(MC - lag, MC)
            ] + [lambda: emit_dance(st)], st

        # fill queue: everything attention unit 0 doesn't need upfront.
        # Ordering matters -- each item must land before its consumer
        # iteration (vh mc needed at unit-0 iter mc+3; khT0 sc by unit-0
        # iter 4*sc; qhT0 sc by unit sc; pair-1 khT/qhT by unit 4+).
        # Unit 0 pops 2 items/iter (32 total) and drains exactly the first
        # 32; later units pop 18 each.
        fill_queue = []
        vq = [lambda mc=mc: emit_proj_v(mc) for mc in range(MC)]
        fill_queue += vq[0:3]
        fill_queue += proj_quarters(0, wk_sb, kT_sb, khT_sb[0], 1)
        fill_queue += vq[3:5]
        fill_queue += proj_quarters(0, wk_sb, kT_sb, khT_sb[0], 2)
        fill_queue += vq[5:7]
        fill_queue += proj_quarters(0, wk_sb, kT_sb, khT_sb[0], 3)
        fill_queue += vq[7:9]
        fill_queue += proj_quarters(0, wq_sb, qT_sb, qhT_sb[0], 1)
        fill_queue += vq[9:16]
        # -- unit 0 boundary (32 items above) --
        # units 1-3 get only what THEY consume (they are ACT-bound anyway);
        # pair-1 khT sc1-3 / qhT1 sc1-3 fill units 4-6, which otherwise
        # starve the PE once the projection work runs out
        fill_queue += proj_quarters(0, wq_sb, qT_sb, qhT_sb[0], 2)   # unit 2
        fill_queue += proj_quarters(1, wk_sb, kT_sb, khT_sb[1], 0)
        fill_queue += proj_quarters(0, wq_sb, qT_sb, qhT_sb[0], 3)   # unit 3
        fill_queue += proj_quarters(1, wq_sb, qT_sb, qhT_sb[1], 0)   # unit 4
        late_fills = []
        for s in range(1, SC):
            late_fills += proj_quarters(1, wk_sb, kT_sb, khT_sb[1], s)
        late_fills += proj_quarters(1, wq_sb, qT_sb, qhT_sb[1], 1)
        late_fills += proj_quarters(1, wq_sb, qT_sb, qhT_sb[1], 2)
        late_fills += proj_quarters(1, wq_sb, qT_sb, qhT_sb[1], 3)

        units = [(p, qc) for p in range(PAIRS) for qc in range(SC)]
        carry = []
        pending, pending2 = [], []
        for u, (p, qc) in enumerate(units):
            if u == 4:
                fill_queue.extend(late_fills)
            carry, last_st = attention_unit(p, qc, fill_queue, carry,
                                            pending, pending2, unit0=(u == 0))
            # outproj for q-range qc becomes legal once unit (1, qc)'s
            # dance has run; that dance is carried into the next unit, so
            # route outproj through pending (joins fills after the carry),
            # split across the unit to keep its tail iterations dense.
            # The last unit's outproj is handled by the chunked drain below.
            pending, pending2 = [], []
            if p == 1 and qc < SC - 1:
                halves = [
                    (lambda qi=qi, j=j: emit_outproj_half(qi, j))
                    for qi in range(qc * 4, qc * 4 + 4)
                    for j in range(2)
                ]
                pending, pending2 = halves[:4], halves[4:]
        # final drain: run the last unit's dance column-chunked, issuing
        # each q-128 range's outproj right after its normalize lands, so
        # the PE never idles (and HAM never re-throttles) during the tail
        while len(carry) > 1:
            carry.pop(0)()  # remaining attn@v
        st = last_st
        p, qc = st["p"], st["qc"]
        pcs = []
        for pout in (st["poutA"], st["poutB"]):
            pc = dance.tile([65, 512], F32, tag="pc", name="pc")
            nc.vector.tensor_copy(pc[:], pout[:])
            pcs.append(pc)
        while fill_queue:
            fill_queue.pop(0)()
        for ci in range(4):
            cols = slice(ci * 128, (ci + 1) * 128)
            for hh, pc in enumerate(pcs):
                hlo, hhi = hh * 64, hh * 64 + 64
                sums = dance.tile([1, 128], F32, tag="sumsf", name="sumsf")
                nc.vector.tensor_copy(sums[:], pc[64:65, cols])
                rcp = dance.tile([1, 128], F32, tag="rcpf", name="rcpf")
                nc.vector.reciprocal_approx_fast(rcp[:], sums[:])
                rcpb = dance.tile([64, 128], F32, tag="rcpbf", name="rcpbf")
                nc.gpsimd.partition_broadcast(rcpb[:], rcp[:])
                nc.vector.tensor_tensor(
                    outT2_sb[p][hlo:hhi, qc * 512 + ci * 128:
                                qc * 512 + (ci + 1) * 128],
                    pc[0:64, cols],
                    rcpb[:],
                    mybir.AluOpType.mult,
                )
            for j in range(2):
                emit_outproj_half(qc * 4 + ci, j)


def build_program():
    nc = bacc.Bacc(
        "TRN2",
        target_bir_lowering=False,
        debug=False,
        enable_asserts=False,
        num_devices=N_CORES,
    )
    qT = nc.dram_tensor("qT", [D, S], BF16, kind="ExternalInput")
    kT = nc.dram_tensor("kT", [D, S], BF16, kind="ExternalInput")
    vT = nc.dram_tensor("vT", [D, S], BF16, kind="ExternalInput")
    wq = nc.dram_tensor("wq", [D, HPC * DK], BF16, kind="ExternalInput")
    wk = nc.dram_tensor("wk", [D, HPC * DK], BF16, kind="ExternalInput")
    wv = nc.dram_tensor("wv", [D, HPC * DK], BF16, kind="ExternalInput")
    wo = nc.dram_tensor("wo", [HPC * DK, D], BF16, kind="ExternalInput")
    out = nc.dram_tensor("out", [S, D], BF16, kind="ExternalOutput")
    with tile.TileContext(nc) as tc:
        _emit(tc, qT, kT, vT, wq, wk, wv, wo, out)
    nc.compile()
    return nc


def _get_program():
    if "nc" not in _COMPILED:
        _COMPILED["nc"] = build_program()
    return _COMPILED["nc"]


def make_in_maps(q, k, v, Wq, Wk, Wv, Wo):
    """Shard FULL fp32 inputs into per-core bf16 input maps."""
    q, k, v = (np.asarray(x, np.float32) for x in (q, k, v))
    Wq, Wk, Wv, Wo = (np.asarray(x, np.float32) for x in (Wq, Wk, Wv, Wo))
    Wq = Wq * (1.0 / np.sqrt(DK))    # fold softmax scale into Wq
    qT = [np.ascontiguousarray(q[b].T).astype(BF16_NP) for b in range(B)]
    kT = [np.ascontiguousarray(k[b].T).astype(BF16_NP) for b in range(B)]
    vT = [np.ascontiguousarray(v[b].T).astype(BF16_NP) for b in range(B)]
    in_maps = []
    for c in range(N_CORES):
        b, g = divmod(c, N_CORES // B)
        heads = range(HPC * g, HPC * g + HPC)
        wq_c = np.concatenate([Wq[h] for h in heads], axis=1).astype(BF16_NP)
        wk_c = np.concatenate([Wk[h] for h in heads], axis=1).astype(BF16_NP)
        wv_c = np.concatenate([Wv[h] for h in heads], axis=1).astype(BF16_NP)
        wo_c = np.concatenate(
            [Wo[h * DK:(h + 1) * DK] for h in heads], axis=0
        ).astype(BF16_NP)
        in_maps.append({
            "qT": qT[b], "kT": kT[b], "vT": vT[b],
            "wq": np.ascontiguousarray(wq_c),
            "wk": np.ascontiguousarray(wk_c),
            "wv": np.ascontiguousarray(wv_c),
            "wo": np.ascontiguousarray(wo_c),
        })
    return in_maps


def run_on_hw(in_maps, trace=False):
    nc = _get_program()
    return bass_utils.run_bass_kernel_spmd(
        nc, in_maps, list(range(N_CORES)), trace=trace
    )


def kernel(q, k, v, Wq, Wk, Wv, Wo, bo):
    in_maps = make_in_maps(q, k, v, Wq, Wk, Wv, Wo)
    res = run_on_hw(in_maps)
    bo = np.asarray(bo, np.float32)
    parts = [np.asarray(r["out"], np.float32) for r in res.results]
    out = np.empty((B, S, D), np.float32)
    per_b = N_CORES // B
    for b in range(B):
        out[b] = np.sum(parts[b * per_b:(b + 1) * per_b], axis=0) + bo
    return out
